# revision 25
# baseline (speedup 1.0000x reference)
"""Domain-adaptive attention on 8 Trainium2 NeuronCores.

Sharding: (batch, head-group) — cores 0-3 take batch 0, cores 4-7 batch 1;
each core owns 3 of the 12 heads. Per core, everything runs in a
"features-on-partitions" transposed layout:

  xT [768, S]    = x[b].T            (natural DMA load + PE transposes)
  qT/kT          = (W slice).T @ xT  (PE, f32r; each head duplicated on both
                                      64-row halves so score matmuls can
                                      alternate PE row-groups -> 2x overlap)
  v natural      = vT transposed back on PE (PV needs tokens on partitions)
  scoresT_j      = kT_j-block.T @ qT ([128 keys x 1024 queries] psum tiles)
  aT_j           = exp(dscale/sqrt(D) * scoresT_j)   (ACT)
  numT/den       = [v | 1].T @ aT_j accumulated      (PE, M=65: row 64 = denom)
  out_projT      = Wp_slice.T-tiles @ (numT/den)     (PE) -> partial [768, S]

Host: computes dscale (tiny), slices weights per core, sums the 4 partial
projections per batch and adds bp. Softmax skips max-subtraction: logits are
bounded (|dscale|<~3, |qk/8|<~6) so exp stays well inside f32 range.
"""

import numpy as np

import concourse.bass as bass
import concourse.mybir as mybir
import concourse.tile as tile
from concourse import bacc
from concourse.bass import ds
from concourse.bass_utils import run_bass_kernel_spmd
from concourse.masks import make_identity

F32 = mybir.dt.float32
F32R = mybir.dt.float32r
AF = mybir.ActivationFunctionType

B, S, E, H, D = 2, 2048, 768, 12, 64
HPC = 3          # heads per core
CPB = 4          # cores per batch
NCORES = 8
KT = E // 128    # 6 contraction tiles for the projections
NB = S // 512    # 4 moving-dim blocks of 512
SJ = S // 128    # 16 key tiles

TRACE = False
LAST_RESULT = None


def build_nc():
    nc = bacc.Bacc(None, target_bir_lowering=False)

    xb = nc.dram_tensor("xb", [S, E], F32R, kind="ExternalInput")
    wq = nc.dram_tensor("wq", [E, HPC * D], F32R, kind="ExternalInput")
    wk = nc.dram_tensor("wk", [E, HPC * D], F32R, kind="ExternalInput")
    wv = nc.dram_tensor("wv", [E, HPC * D], F32R, kind="ExternalInput")
    wp = nc.dram_tensor("wp", [HPC * D, E], F32R, kind="ExternalInput")
    scl = nc.dram_tensor("scl", [128, HPC], F32, kind="ExternalInput")
    bqkv = nc.dram_tensor("bqkv", [128, 6], F32, kind="ExternalInput")
    outp = nc.dram_tensor("outp", [E, S], F32, kind="ExternalOutput")

    # M-tiles over the 192 per-core feature columns
    MT = [(0, 128), (128, 64)]
    # per-head home slot: (plane, row offset) in qT/kT/pIn
    HEAD_RO = [(0, 0), (1, 64), (2, 0)]

    with tile.TileContext(nc) as tc:
        with (
            tc.tile_pool(name="persist", bufs=1) as pp,
            tc.tile_pool(name="vstage", bufs=4) as vsp,
        ):
            # ---- persistent SBUF tiles ----
            wq_sb = pp.tile([128, KT, HPC * D], F32R, tag="wq_sb")
            wk_sb = pp.tile([128, KT, HPC * D], F32R, tag="wk_sb")
            wv_sb = pp.tile([128, KT, HPC * D], F32R, tag="wv_sb")
            wp_sb = pp.tile([128, 2, E], F32R, tag="wp_sb")
            scl_sb = pp.tile([128, HPC], F32, tag="scl_sb")
            bqkv_sb = pp.tile([128, 6], F32, tag="bqkv_sb")
            # per-head planes; each head's 64 rows duplicated into both halves
            qT = pp.tile([128, HPC, S], F32R, tag="qT")
            kT = pp.tile([128, HPC, S], F32R, tag="kT")
            v_sb = pp.tile([128, HPC, SJ, D + 1], F32R, tag="v_sb")
            pIn = pp.tile([128, 2, S], F32R, tag="pIn")
            ident = pp.tile([128, 128], F32, tag="ident")
            identr = pp.tile([128, 128], F32R, tag="identr")

            # ---- loads ----
            nc.sync.dma_start(out=wq_sb, in_=wq.rearrange("(ko p) m -> p ko m", p=128))
            nc.sync.dma_start(out=wk_sb, in_=wk.rearrange("(ko p) m -> p ko m", p=128))
            nc.sync.dma_start(out=wv_sb, in_=wv.rearrange("(ko p) m -> p ko m", p=128))
            nc.sync.dma_start(out=wp_sb[:, 0, :], in_=wp[0:128, :])
            nc.sync.dma_start(out=wp_sb[0:64, 1, :], in_=wp[128:192, :])
            nc.sync.dma_start(out=scl_sb, in_=scl[:, :])
            nc.sync.dma_start(out=bqkv_sb, in_=bqkv[:, :])
            make_identity(nc, ident)
            nc.vector.tensor_copy(identr, ident)
            ones_f = pp.tile([128, 1], F32, tag="ones_f")
            nc.vector.memset(ones_f, 1.0)
            for h in range(HPC):
                for j in range(SJ):
                    nc.vector.tensor_copy(v_sb[:, h, j, D : D + 1], ones_f)
            ones64 = pp.tile([1, 64], F32R, tag="ones64")
            nc.vector.tensor_copy(ones64, ones_f[0:1, 0:1].broadcast_to([1, 64]))

            with (
                tc.tile_pool(name="xtp", bufs=1) as xp,
                tc.tile_pool(name="pj_ps", bufs=4, space="PSUM") as pjps,
                tc.tile_pool(name="vt_ps", bufs=2, space="PSUM") as vtps,
            ):
                # xt split by 512-query blocks so QKV can start before all
                # transposes finish
                xts = [
                    xp.tile([128, KT, 512], F32R, tag=f"xt{nb}", name=f"xt{nb}")
                    for nb in range(NB)
                ]
                with tc.tile_pool(name="xn", bufs=3) as xnp:
                    for t in range(SJ):
                        xn = xnp.tile([128, E], F32R, tag="xn")
                        nc.sync.dma_start(out=xn, in_=xb[ds(t * 128, 128), :])
                        for ko in range(KT):
                            tp = vtps.tile([128, 128], F32R, tag="tp")
                            nc.tensor.transpose(
                                out=tp,
                                in_=xn[:, ds(ko * 128, 128)],
                                identity=identr,
                            )
                            nc.vector.tensor_copy(
                                xts[t // 4][:, ko, ds((t % 4) * 128, 128)], tp
                            )

                # ---- q/k projections (transposed layout) ----
                # psum mt0 rows 0-63 = head0, rows 64-127 = head1; mt1 = head2
                for w_sb, dst, bcol in ((wq_sb, qT, 0), (wk_sb, kT, 2)):
                    for nb in range(NB):
                        for mt, (coff, csz) in enumerate(MT):
                            pt = pjps.tile([128, 512], F32, tag="pt")
                            for ko in range(KT):
                                nc.tensor.matmul(
                                    pt[0:csz, :],
                                    lhsT=w_sb[:, ko, coff : coff + csz],
                                    rhs=xts[nb][:, ko, :],
                                    start=(ko == 0),
                                    stop=(ko == KT - 1),
                                )
                            if mt == 0:
                                nc.vector.tensor_scalar_add(
                                    out=dst[0:64, 0, ds(nb * 512, 512)],
                                    in0=pt[0:64, :],
                                    scalar1=bqkv_sb[0:64, bcol : bcol + 1],
                                )
                                nc.vector.tensor_scalar_add(
                                    out=dst[64:128, 1, ds(nb * 512, 512)],
                                    in0=pt[64:128, :],
                                    scalar1=bqkv_sb[64:128, bcol : bcol + 1],
                                )
                            else:
                                nc.vector.tensor_scalar_add(
                                    out=dst[0:64, 2, ds(nb * 512, 512)],
                                    in0=pt[0:64, :],
                                    scalar1=bqkv_sb[0:64, bcol + 1 : bcol + 2],
                                )

                # ---- v projection + transpose back to natural layout ----
                for nb in range(NB):
                    for mt, (coff, csz) in enumerate(MT):
                        pt = pjps.tile([128, 512], F32, tag="pt")
                        for ko in range(KT):
                            nc.tensor.matmul(
                                pt[0:csz, :],
                                lhsT=wv_sb[:, ko, coff : coff + csz],
                                rhs=xts[nb][:, ko, :],
                                start=(ko == 0),
                                stop=(ko == KT - 1),
                            )
                        for jj in range(4):
                            j = nb * 4 + jj
                            for hh in range(csz // D):
                                h = mt * 2 + hh
                                stg = vsp.tile([D, 128], F32R, tag="vstg")
                                nc.vector.tensor_scalar_add(
                                    out=stg,
                                    in0=pt[ds(hh * D, D), ds(jj * 128, 128)],
                                    scalar1=bqkv_sb[ds(hh * D, D), 4 + mt : 5 + mt],
                                )
                                pvt = vtps.tile([128, D], F32R, tag="pvt")
                                nc.tensor.transpose(
                                    out=pvt[:, :],
                                    in_=stg,
                                    identity=identr[0:D, 0:D],
                                )
                                nc.vector.tensor_copy(v_sb[:, h, j, 0:D], pvt[:, :])

            # duplicate each head's q/k rows into the other 64-row half so its
            # score matmuls can alternate PE row-groups (2x concurrency)
            for t_ in (qT, kT):
                nc.sync.dma_start(out=t_[64:128, 0, :], in_=t_[0:64, 0, :])
                nc.sync.dma_start(out=t_[0:64, 1, :], in_=t_[64:128, 1, :])
                nc.sync.dma_start(out=t_[64:128, 2, :], in_=t_[0:64, 2, :])

            # ---- attention, transposed orientation ----
            with (
                tc.tile_pool(name="att", bufs=4) as atp,
                tc.tile_pool(name="norm", bufs=2) as nrp,
                tc.tile_pool(name="sc_ps", bufs=3, space="PSUM") as scps,
                tc.tile_pool(name="o_ps", bufs=1, space="PSUM") as ops,
            ):

                def normalize(h, half, po):
                    pl, ro = HEAD_RO[h][0] // 2, HEAD_RO[h][1]
                    pl = 0 if h < 2 else 1
                    den = nrp.tile([1, 1024], F32, tag="den")
                    nc.vector.tensor_copy(den, po[D : D + 1, :])
                    rc = nrp.tile([1, 1024], F32, tag="rc")
                    rscr = nrp.tile([1, 1024], F32, tag="rscr")
                    nc.vector.reciprocal_approx_accurate(out=rc, in_=den, scratch=rscr)
                    rcr = nrp.tile([1, 1024], F32R, tag="rcr")
                    nc.vector.tensor_copy(rcr, rc)
                    pn = nrp.tile([64, 1024], F32, tag="pn")
                    nc.vector.tensor_copy(pn, po[0:D, :])
                    for q2 in range(2):
                        pb = scps.tile([64, 512], F32, tag="ps")
                        nc.tensor.matmul(
                            pb,
                            lhsT=ones64,
                            rhs=rcr[:, ds(q2 * 512, 512)],
                            start=True,
                            stop=True,
                        )
                        nc.vector.tensor_mul(
                            pIn[ro : ro + 64, pl, ds(half * 1024 + q2 * 512, 512)],
                            pn[:, ds(q2 * 512, 512)],
                            pb,
                        )

                for h in range(HPC):
                    for half in range(2):
                        po = ops.tile([D + 1, 1024], F32, tag="po")
                        for j in range(SJ):
                            ps = scps.tile([128, 1024], F32, tag="ps")
                            for q2 in range(2):
                                ro2 = 64 * q2
                                qoff = half * 1024 + q2 * 512
                                nc.tensor.matmul(
                                    ps[:, ds(q2 * 512, 512)],
                                    lhsT=kT[ro2 : ro2 + 64, h, ds(j * 128, 128)],
                                    rhs=qT[ro2 : ro2 + 64, h, ds(qoff, 512)],
                                    start=True,
                                    stop=True,
                                )
                            at = atp.tile([128, 1024], F32R, tag="at")
                            nc.scalar.activation(
                                out=at, in_=ps, func=AF.Exp, scale=scl_sb[:, h : h + 1]
                            )
                            for q2 in range(2):
                                nc.tensor.matmul(
                                    po[:, ds(q2 * 512, 512)],
                                    lhsT=v_sb[:, h, j, :],
                                    rhs=at[:, ds(q2 * 512, 512)],
                                    start=(j == 0),
                                    stop=(j == SJ - 1),
                                )
                        normalize(h, half, po)

            # ---- output projection (partial; summed across cores on host) ----
            with (
                tc.tile_pool(name="prout", bufs=6) as prp,
                tc.tile_pool(name="op_ps", bufs=6, space="PSUM") as opps,
            ):
                for mt in range(6):
                    for nb in range(NB):
                        pr = opps.tile([128, 512], F32, tag="pr")
                        nc.tensor.matmul(
                            pr,
                            lhsT=wp_sb[:, 0, ds(mt * 128, 128)],
                            rhs=pIn[:, 0, ds(nb * 512, 512)],
                            start=True,
                            stop=False,
                        )
                        nc.tensor.matmul(
                            pr,
                            lhsT=wp_sb[0:64, 1, ds(mt * 128, 128)],
                            rhs=pIn[0:64, 1, ds(nb * 512, 512)],
                            start=False,
                            stop=True,
                        )
                        prs = prp.tile([128, 512], F32, tag="prs")
                        nc.vector.tensor_copy(prs, pr)
                        nc.sync.dma_start(
                            out=outp[ds(mt * 128, 128), ds(nb * 512, 512)], in_=prs
                        )

    nc.compile()
    return nc


_NC = None


def _get_nc():
    global _NC
    if _NC is None:
        _NC = build_nc()
    return _NC


def make_in_maps(x, domain_embedding, Wq, bq, Wk, bk, Wv, bv, Wd, bd, Wp, bp):
    f = lambda a: np.ascontiguousarray(np.asarray(a, dtype=np.float32))
    x, domain_embedding = f(x), f(domain_embedding)
    Wq, Wk, Wv, Wp, Wd = f(Wq), f(Wk), f(Wv), f(Wp), f(Wd)
    bq, bk, bv, bd = f(bq), f(bk), f(bv), f(bd)

    dscale = domain_embedding @ Wd + bd  # [B, H]
    in_maps = []
    for c in range(NCORES):
        b, h0 = c // CPB, HPC * (c % CPB)
        cols = slice(D * h0, D * h0 + HPC * D)
        bqkv_np = np.zeros((128, 6), np.float32)
        for i, bias in enumerate((bq, bk, bv)):
            bqkv_np[:, 2 * i] = bias[cols][0:128]
            bqkv_np[0:64, 2 * i + 1] = bias[cols][128:192]
        scl_np = np.tile(
            (dscale[b, h0 : h0 + HPC] * (1.0 / np.sqrt(D))).astype(np.float32)[None, :],
            (128, 1),
        )
        in_maps.append(
            {
                "xb": f(x[b]),
                "wq": f(Wq[:, cols]),
                "wk": f(Wk[:, cols]),
                "wv": f(Wv[:, cols]),
                "wp": f(Wp[cols, :]),
                "scl": np.ascontiguousarray(scl_np),
                "bqkv": bqkv_np,
            }
        )
    return in_maps


def kernel(x, domain_embedding, Wq, bq, Wk, bk, Wv, bv, Wd, bd, Wp, bp):
    global LAST_RESULT
    in_maps = make_in_maps(
        x, domain_embedding, Wq, bq, Wk, bk, Wv, bv, Wd, bd, Wp, bp
    )
    res = run_bass_kernel_spmd(
        _get_nc(), in_maps, core_ids=list(range(NCORES)), trace=TRACE
    )
    LAST_RESULT = res
    bp = np.asarray(bp, dtype=np.float32)
    out = np.empty((B, S, E), np.float32)
    for b in range(B):
        acc = res.results[CPB * b]["outp"].copy()
        for c in range(CPB * b + 1, CPB * (b + 1)):
            acc += res.results[c]["outp"]
        out[b] = acc.T + bp[None, :]
    return out


# revision 27
# speedup vs baseline: 1.1299x; 1.1299x over previous
"""Domain-adaptive attention on 8 Trainium2 NeuronCores.

Sharding: (batch, head-group) — cores 0-3 take batch 0, cores 4-7 batch 1;
each core owns 3 of the 12 heads. Per core, everything runs in a
"features-on-partitions" transposed layout:

  xT [768, S]    = x[b].T            (natural DMA load + PE transposes)
  qT/kT          = (W slice).T @ xT  (PE, f32r; each head duplicated on both
                                      64-row halves so score matmuls can
                                      alternate PE row-groups -> 2x overlap)
  v natural      = vT transposed back on PE (PV needs tokens on partitions)
  scoresT_j      = kT_j-block.T @ qT ([128 keys x 1024 queries] psum tiles)
  aT_j           = exp(dscale/sqrt(D) * scoresT_j)   (ACT)
  numT/den       = [v | 1].T @ aT_j accumulated      (PE, M=65: row 64 = denom)
  out_projT      = Wp_slice.T-tiles @ (numT/den)     (PE) -> partial [768, S]

Host: computes dscale (tiny), slices weights per core, sums the 4 partial
projections per batch and adds bp. Softmax skips max-subtraction: logits are
bounded (|dscale|<~3, |qk/8|<~6) so exp stays well inside f32 range.
"""

import numpy as np

import concourse.bass as bass
import concourse.mybir as mybir
import concourse.tile as tile
from concourse import bacc
from concourse.bass import ds
from concourse.bass_utils import run_bass_kernel_spmd
from concourse.masks import make_identity

F32 = mybir.dt.float32
F32R = mybir.dt.float32r
AF = mybir.ActivationFunctionType

B, S, E, H, D = 2, 2048, 768, 12, 64
HPC = 3          # heads per core
CPB = 4          # cores per batch
NCORES = 8
KT = E // 128    # 6 contraction tiles for the projections
NB = S // 512    # 4 moving-dim blocks of 512
SJ = S // 128    # 16 key tiles

TRACE = False
LAST_RESULT = None


def build_nc():
    nc = bacc.Bacc(None, target_bir_lowering=False)

    xb = nc.dram_tensor("xb", [S, E], F32R, kind="ExternalInput")
    wq = nc.dram_tensor("wq", [E, HPC * D], F32R, kind="ExternalInput")
    wk = nc.dram_tensor("wk", [E, HPC * D], F32R, kind="ExternalInput")
    wv = nc.dram_tensor("wv", [E, HPC * D], F32R, kind="ExternalInput")
    wp = nc.dram_tensor("wp", [HPC * D, E], F32R, kind="ExternalInput")
    scl = nc.dram_tensor("scl", [128, HPC], F32, kind="ExternalInput")
    bqkv = nc.dram_tensor("bqkv", [128, 6], F32, kind="ExternalInput")
    outp = nc.dram_tensor("outp", [E, S], F32, kind="ExternalOutput")

    # M-tiles over the 192 per-core feature columns
    MT = [(0, 128), (128, 64)]
    # per-head home slot: (plane, row offset) in qT/kT/pIn
    HEAD_RO = [(0, 0), (1, 64), (2, 0)]

    with tile.TileContext(nc) as tc:
        with (
            tc.tile_pool(name="persist", bufs=1) as pp,
            tc.tile_pool(name="vstage", bufs=4) as vsp,
        ):
            # ---- persistent SBUF tiles ----
            wq_sb = pp.tile([128, KT, HPC * D], F32R, tag="wq_sb")
            wk_sb = pp.tile([128, KT, HPC * D], F32R, tag="wk_sb")
            wv_sb = pp.tile([128, KT, HPC * D], F32R, tag="wv_sb")
            wp_sb = pp.tile([128, 2, E], F32R, tag="wp_sb")
            scl_sb = pp.tile([128, HPC], F32, tag="scl_sb")
            bqkv_sb = pp.tile([128, 6], F32, tag="bqkv_sb")
            # per-head planes; each head's 64 rows duplicated into both halves
            qT = pp.tile([128, HPC, S], F32R, tag="qT")
            kT = pp.tile([128, HPC, S], F32R, tag="kT")
            v_sb = pp.tile([128, HPC, SJ, D + 1], F32R, tag="v_sb")
            pIn = pp.tile([128, 2, S], F32R, tag="pIn")
            ident = pp.tile([128, 128], F32, tag="ident")
            identr = pp.tile([128, 128], F32R, tag="identr")

            # ---- loads ----
            nc.sync.dma_start(out=wq_sb, in_=wq.rearrange("(ko p) m -> p ko m", p=128))
            nc.sync.dma_start(out=wk_sb, in_=wk.rearrange("(ko p) m -> p ko m", p=128))
            nc.sync.dma_start(out=wv_sb, in_=wv.rearrange("(ko p) m -> p ko m", p=128))
            nc.sync.dma_start(out=wp_sb[:, 0, :], in_=wp[0:128, :])
            nc.sync.dma_start(out=wp_sb[0:64, 1, :], in_=wp[128:192, :])
            nc.sync.dma_start(out=scl_sb, in_=scl[:, :])
            nc.sync.dma_start(out=bqkv_sb, in_=bqkv[:, :])
            make_identity(nc, ident)
            nc.vector.tensor_copy(identr, ident)
            ones_f = pp.tile([128, 1], F32, tag="ones_f")
            nc.vector.memset(ones_f, 1.0)
            for h in range(HPC):
                for j in range(SJ):
                    nc.vector.tensor_copy(v_sb[:, h, j, D : D + 1], ones_f)
            ones64 = pp.tile([1, 64], F32R, tag="ones64")
            nc.vector.tensor_copy(ones64, ones_f[0:1, 0:1].broadcast_to([1, 64]))

            with (
                tc.tile_pool(name="xtp", bufs=1) as xp,
                tc.tile_pool(name="pj_ps", bufs=4, space="PSUM") as pjps,
                tc.tile_pool(name="vt_ps", bufs=2, space="PSUM") as vtps,
            ):
                # xt split by 512-query blocks so QKV can start before all
                # transposes finish
                xts = [
                    xp.tile([128, KT, 512], F32R, tag=f"xt{nb}", name=f"xt{nb}")
                    for nb in range(NB)
                ]
                with tc.tile_pool(name="xn", bufs=3) as xnp:
                    for t in range(SJ):
                        xn = xnp.tile([128, E], F32R, tag="xn")
                        nc.sync.dma_start(out=xn, in_=xb[ds(t * 128, 128), :])
                        for ko in range(KT):
                            tp = vtps.tile([128, 128], F32R, tag="tp")
                            nc.tensor.transpose(
                                out=tp,
                                in_=xn[:, ds(ko * 128, 128)],
                                identity=identr,
                            )
                            nc.vector.tensor_copy(
                                xts[t // 4][:, ko, ds((t % 4) * 128, 128)], tp
                            )

                # ---- q/k projections (transposed layout) ----
                # psum mt0 rows 0-63 = head0, rows 64-127 = head1; mt1 = head2
                for w_sb, dst, bcol in ((wq_sb, qT, 0), (wk_sb, kT, 2)):
                    for nb in range(NB):
                        for mt, (coff, csz) in enumerate(MT):
                            pt = pjps.tile([128, 512], F32, tag="pt")
                            for ko in range(KT):
                                nc.tensor.matmul(
                                    pt[0:csz, :],
                                    lhsT=w_sb[:, ko, coff : coff + csz],
                                    rhs=xts[nb][:, ko, :],
                                    start=(ko == 0),
                                    stop=(ko == KT - 1),
                                )
                            if mt == 0:
                                nc.vector.tensor_scalar_add(
                                    out=dst[0:64, 0, ds(nb * 512, 512)],
                                    in0=pt[0:64, :],
                                    scalar1=bqkv_sb[0:64, bcol : bcol + 1],
                                )
                                nc.vector.tensor_scalar_add(
                                    out=dst[64:128, 1, ds(nb * 512, 512)],
                                    in0=pt[64:128, :],
                                    scalar1=bqkv_sb[64:128, bcol : bcol + 1],
                                )
                            else:
                                nc.vector.tensor_scalar_add(
                                    out=dst[0:64, 2, ds(nb * 512, 512)],
                                    in0=pt[0:64, :],
                                    scalar1=bqkv_sb[0:64, bcol + 1 : bcol + 2],
                                )

                # ---- v projection + transpose back to natural layout ----
                for nb in range(NB):
                    for mt, (coff, csz) in enumerate(MT):
                        pt = pjps.tile([128, 512], F32, tag="pt")
                        for ko in range(KT):
                            nc.tensor.matmul(
                                pt[0:csz, :],
                                lhsT=wv_sb[:, ko, coff : coff + csz],
                                rhs=xts[nb][:, ko, :],
                                start=(ko == 0),
                                stop=(ko == KT - 1),
                            )
                        for jj in range(4):
                            j = nb * 4 + jj
                            for hh in range(csz // D):
                                h = mt * 2 + hh
                                stg = vsp.tile([D, 128], F32R, tag="vstg")
                                nc.vector.tensor_scalar_add(
                                    out=stg,
                                    in0=pt[ds(hh * D, D), ds(jj * 128, 128)],
                                    scalar1=bqkv_sb[ds(hh * D, D), 4 + mt : 5 + mt],
                                )
                                pvt = vtps.tile([128, D], F32R, tag="pvt")
                                nc.tensor.transpose(
                                    out=pvt[:, :],
                                    in_=stg,
                                    identity=identr[0:D, 0:D],
                                )
                                nc.vector.tensor_copy(v_sb[:, h, j, 0:D], pvt[:, :])

            # duplicate each head's q/k rows into the other 64-row half so its
            # score matmuls can alternate PE row-groups (2x concurrency)
            for t_ in (qT, kT):
                nc.sync.dma_start(out=t_[64:128, 0, :], in_=t_[0:64, 0, :])
                nc.sync.dma_start(out=t_[0:64, 1, :], in_=t_[64:128, 1, :])
                nc.sync.dma_start(out=t_[64:128, 2, :], in_=t_[0:64, 2, :])

            # ---- attention, transposed orientation ----
            with (
                tc.tile_pool(name="att", bufs=4) as atp,
                tc.tile_pool(name="norm", bufs=1) as nrp,
                tc.tile_pool(name="sc_ps", bufs=2, space="PSUM") as scps,
                tc.tile_pool(name="o_ps", bufs=2, space="PSUM") as ops,
            ):

                def normalize(h, half, po):
                    pl, ro = HEAD_RO[h][0] // 2, HEAD_RO[h][1]
                    pl = 0 if h < 2 else 1
                    den = nrp.tile([1, 1024], F32, tag="den")
                    nc.vector.tensor_copy(den, po[D : D + 1, :])
                    rc = nrp.tile([1, 1024], F32, tag="rc")
                    nc.vector.reciprocal_approx_fast(out=rc, in_=den)
                    rcr = nrp.tile([1, 1024], F32R, tag="rcr")
                    nc.vector.tensor_copy(rcr, rc)
                    pn = nrp.tile([64, 1024], F32, tag="pn")
                    nc.vector.tensor_copy(pn, po[0:D, :])
                    for q2 in range(2):
                        pb = ops.tile([64, 512], F32, tag="po")
                        nc.tensor.matmul(
                            pb,
                            lhsT=ones64,
                            rhs=rcr[:, ds(q2 * 512, 512)],
                            start=True,
                            stop=True,
                        )
                        nc.vector.tensor_mul(
                            pIn[ro : ro + 64, pl, ds(half * 1024 + q2 * 512, 512)],
                            pn[:, ds(q2 * 512, 512)],
                            pb,
                        )

                def attend(h):
                    for half in range(2):
                        po = ops.tile([D + 1, 1024], F32, tag="po")
                        for j in range(SJ):
                            ps = scps.tile([128, 1024], F32, tag="ps")
                            for q2 in range(2):
                                ro2 = 64 * q2
                                qoff = half * 1024 + q2 * 512
                                nc.tensor.matmul(
                                    ps[:, ds(q2 * 512, 512)],
                                    lhsT=kT[ro2 : ro2 + 64, h, ds(j * 128, 128)],
                                    rhs=qT[ro2 : ro2 + 64, h, ds(qoff, 512)],
                                    start=True,
                                    stop=True,
                                )
                            at = atp.tile([128, 1024], F32R, tag="at")
                            nc.scalar.activation(
                                out=at, in_=ps, func=AF.Exp, scale=scl_sb[:, h : h + 1]
                            )
                            for q2 in range(2):
                                nc.tensor.matmul(
                                    po[:, ds(q2 * 512, 512)],
                                    lhsT=v_sb[:, h, j, :],
                                    rhs=at[:, ds(q2 * 512, 512)],
                                    start=(j == 0),
                                    stop=(j == SJ - 1),
                                )
                        normalize(h, half, po)

                with (
                    tc.tile_pool(name="prstage", bufs=1) as prsp,
                    tc.tile_pool(name="prout", bufs=4) as prp,
                ):
                    pr0 = prsp.tile([128, 6, S], F32, tag="pr0")

                    attend(0)
                    attend(1)
                    # plane-0 projection pass overlaps head-2 attention
                    for mt in range(6):
                        for nb in range(NB):
                            pr = scps.tile([128, 512], F32, tag="ps")
                            nc.tensor.matmul(
                                pr,
                                lhsT=wp_sb[:, 0, ds(mt * 128, 128)],
                                rhs=pIn[:, 0, ds(nb * 512, 512)],
                                start=True,
                                stop=True,
                            )
                            nc.vector.tensor_copy(
                                pr0[:, mt, ds(nb * 512, 512)], pr
                            )
                    attend(2)
                    # plane-1 residual + combine + store
                    for mt in range(6):
                        for nb in range(NB):
                            pr = scps.tile([128, 512], F32, tag="ps")
                            nc.tensor.matmul(
                                pr,
                                lhsT=wp_sb[0:64, 1, ds(mt * 128, 128)],
                                rhs=pIn[0:64, 1, ds(nb * 512, 512)],
                                start=True,
                                stop=True,
                            )
                            prs = prp.tile([128, 512], F32, tag="prs")
                            nc.vector.tensor_add(
                                prs, pr, pr0[:, mt, ds(nb * 512, 512)]
                            )
                            nc.sync.dma_start(
                                out=outp[ds(mt * 128, 128), ds(nb * 512, 512)],
                                in_=prs,
                            )

    nc.compile()
    return nc


_NC = None


def _get_nc():
    global _NC
    if _NC is None:
        _NC = build_nc()
    return _NC


def make_in_maps(x, domain_embedding, Wq, bq, Wk, bk, Wv, bv, Wd, bd, Wp, bp):
    f = lambda a: np.ascontiguousarray(np.asarray(a, dtype=np.float32))
    x, domain_embedding = f(x), f(domain_embedding)
    Wq, Wk, Wv, Wp, Wd = f(Wq), f(Wk), f(Wv), f(Wp), f(Wd)
    bq, bk, bv, bd = f(bq), f(bk), f(bv), f(bd)

    dscale = domain_embedding @ Wd + bd  # [B, H]
    in_maps = []
    for c in range(NCORES):
        b, h0 = c // CPB, HPC * (c % CPB)
        cols = slice(D * h0, D * h0 + HPC * D)
        bqkv_np = np.zeros((128, 6), np.float32)
        for i, bias in enumerate((bq, bk, bv)):
            bqkv_np[:, 2 * i] = bias[cols][0:128]
            bqkv_np[0:64, 2 * i + 1] = bias[cols][128:192]
        scl_np = np.tile(
            (dscale[b, h0 : h0 + HPC] * (1.0 / np.sqrt(D))).astype(np.float32)[None, :],
            (128, 1),
        )
        in_maps.append(
            {
                "xb": f(x[b]),
                "wq": f(Wq[:, cols]),
                "wk": f(Wk[:, cols]),
                "wv": f(Wv[:, cols]),
                "wp": f(Wp[cols, :]),
                "scl": np.ascontiguousarray(scl_np),
                "bqkv": bqkv_np,
            }
        )
    return in_maps


def kernel(x, domain_embedding, Wq, bq, Wk, bk, Wv, bv, Wd, bd, Wp, bp):
    global LAST_RESULT
    in_maps = make_in_maps(
        x, domain_embedding, Wq, bq, Wk, bk, Wv, bv, Wd, bd, Wp, bp
    )
    res = run_bass_kernel_spmd(
        _get_nc(), in_maps, core_ids=list(range(NCORES)), trace=TRACE
    )
    LAST_RESULT = res
    bp = np.asarray(bp, dtype=np.float32)
    out = np.empty((B, S, E), np.float32)
    for b in range(B):
        acc = res.results[CPB * b]["outp"].copy()
        for c in range(CPB * b + 1, CPB * (b + 1)):
            acc += res.results[c]["outp"]
        out[b] = acc.T + bp[None, :]
    return out
